# revision 35
# baseline (speedup 1.0000x reference)
"""GQA attention kernel for Trainium2, 8-core sequence-parallel SPMD.

Model: d_model=1024, 16 q-heads / 4 kv-heads of dim 64, seq 4096, batch 1.

Per-core split: core c handles query rows [512c, 512c+512) for ALL 16 heads.

v5 design ("fully replicated projections -- zero collectives"):
  - EVERY core streams the full x^T (fp16, block-buffered DMA) and computes
    the complete kT AND v for all 4096 keys itself (~140k extra PE cycles).
    Measured on this part, collectives cost 60-105us of latency (launch-skew
    barrier + RDH transfer) that no amount of overlap could hide, while the
    exp stream (ScalarE+VectorE, ~150us/engine of PSUM-bound streaming) must
    start as early as possible.  Replication makes the kernel fully
    deterministic: no barrier, no skew sensitivity, no gather scatter.
  - Front region: per key-block b, kproj(b) + vproj(b) are interleaved with
    the scores+exp of slots 0 and 1 for that block, so the exp engines are
    fed from ~15us while the PE grinds projections.
  - Global cross-slot AV pipeline (FIFO, lag 6, <=2 pops per step): slot
    s's tail AVs emit after slot s+1's early scores so an exp-wait never
    head-of-line blocks ready scores work in the in-order PE queue, and the
    6+ ready AVs buffer the PE across each slot boundary (prevents the HAM
    activity re-throttle that an idle gap >3.4us causes).
  - qproj(s) for s>=2 rides inside the previous steady slot's PE slack.
  - exp split across ScalarE (activation Exp -> fp8 direct) and VectorE
    (ONE tensor_scalar: int8(score*log2e + 55.65) bit pattern IS the
    fp8e4m3 value of exp(score/8) -- magic-number exp2 at 3 mantissa bits;
    its systematic bias cancels in softmax normalization).
  - normalize: denominator row moved by sb2sb DMA (off the exp engines),
    gpsimd broadcast, DVE reciprocal, gpsimd multiplies; per-slot ctxt
    tiles give the output projection exact per-slot deps so it overlaps
    slot 7's normalize chain.
  - All bulk input DMAs ride the sync (SP) HWDGE ring so DMA-issue
    instructions never steal ScalarE compute time.
"""

import sys
import numpy as np

sys.path.insert(0, "/opt/trn_rl_repo")

from contextlib import ExitStack  # noqa: E402

import concourse.bass as bass  # noqa: E402
import concourse.bacc as bacc  # noqa: E402
import concourse.tile as tile  # noqa: E402
from concourse import mybir  # noqa: E402
from concourse.bass_utils import run_bass_kernel_spmd  # noqa: E402

N_CORES = 8
SEQ = 4096
DM = 1024
QS = SEQ // N_CORES  # 512 query rows per core
HD = 64
NQ = 16
NKV = 4
KV = NKV * HD  # 256
CC = DM // 128  # 8 contraction chunks
KC = SEQ // 128  # 32 key chunks
QT = QS // 128  # 4 query row tiles
F16 = mybir.dt.float16
F32 = mybir.dt.float32
I8 = mybir.dt.int8
F8 = mybir.dt.float8e4
ts = bass.ts

# DVE fast-exp2 straight to fp8e4m3 bits:
#   at2_bits = int8(score * EXP_SCALE8 + EXP_OFFSET8)
EXP_SCALE8 = float(np.log2(np.e))  # folds the 0.125 softmax scale and 2**3
EXP_OFFSET8 = float((7.0 - 0.0434) * 8.0)

# Per-slot engine split for the 32 exp tiles: True -> DVE int8-magic path.
DVE_N = 14
DVE_TILE = [(i * DVE_N) // KC != ((i + 1) * DVE_N) // KC for i in range(KC)]
AV_LAG = 6

_CACHE = {}
DEBUG_DUMP = False


def _emit(tc: tile.TileContext):
    nc = tc.nc
    # All inputs pre-laid-out on host, fp16.
    xot = nc.dram_tensor("xot", [128, CC, QS], F16, kind="ExternalInput").ap()
    xst = nc.dram_tensor("xst", [128, CC, SEQ], F16, kind="ExternalInput").ap()
    Wq = nc.dram_tensor("wq", [128, CC, DM], F16, kind="ExternalInput").ap()
    bq2 = nc.dram_tensor("bq2", [128, CC], F32, kind="ExternalInput").ap()
    Wk = nc.dram_tensor("wk", [128, CC, KV], F16, kind="ExternalInput").ap()
    bk2 = nc.dram_tensor("bk2", [128, 2], F32, kind="ExternalInput").ap()
    Wv = nc.dram_tensor("wv", [128, CC, KV], F16, kind="ExternalInput").ap()
    bv = nc.dram_tensor("bv", [1, KV], F16, kind="ExternalInput").ap()
    Wo = nc.dram_tensor("wo", [128, CC, DM], F16, kind="ExternalInput").ap()
    bo = nc.dram_tensor("bo", [1, DM], F16, kind="ExternalInput").ap()
    out = nc.dram_tensor("out", [QS, DM], F32, kind="ExternalOutput").ap()

    stack = ExitStack()
    with stack:
        consts = stack.enter_context(tc.tile_pool(name="consts", bufs=1))
        wk_sb = consts.tile([128, CC, KV], F16)
        bk_sb = consts.tile([128, 2], F32)
        wv_sb = consts.tile([128, CC, KV], F16)
        bv_sb = consts.tile([1, KV], F16)
        wq_sb = consts.tile([128, CC, DM], F16)
        bq_sb = consts.tile([128, CC], F32)
        wo_sb = consts.tile([128, CC, DM], F16)
        bo_sb = consts.tile([1, DM], F16)
        ones_sb = consts.tile([1, 512], F16)
        nc.vector.memset(ones_sb[:], 1.0)

        # persistent activations
        acts = stack.enter_context(tc.tile_pool(name="acts", bufs=1))
        xot_sb = acts.tile([128, CC, QS], F16)
        kt0_sb = acts.tile([128, SEQ], F16)        # kv pair 0 dims x seq
        kt1_sb = acts.tile([128, SEQ], F16)        # kv pair 1 dims x seq
        # v per block: [kcp-in-block, parity, kv head 0..3, d(+1 ones)] fp8
        v_sbs = [
            acts.tile([128, 2, 2, 4, 72], F8, name=f"v{b}") for b in range(8)
        ]
        qt_sb = acts.tile([128, CC, QS], F16)      # shuffled q dims x q-rows
        ctxt_sbs = [acts.tile([128, QS], F16, name=f"ctxt{s}") for s in range(8)]
        for b in range(8):
            nc.gpsimd.memset(v_sbs[b][:, :, :, :, HD], 1.0)

        # streamed full-x blocks for the replicated k/v projections
        xsp = stack.enter_context(tc.tile_pool(name="xs", bufs=3))

        # All bulk loads on the sync (SP) ring, ordered so transfers stay
        # ahead of their consumers.
        nc.sync.dma_start(bv_sb[:], bv)
        nc.sync.dma_start(bk_sb[:], bk2)
        nc.sync.dma_start(bq_sb[:], bq2)
        nc.sync.dma_start(wv_sb[:], Wv)
        nc.sync.dma_start(wk_sb[:], Wk)
        xs_tiles = [
            xsp.tile([128, CC, QS], F16, tag="xs", name=f"xs{b}") for b in range(8)
        ]
        nc.sync.dma_start(xs_tiles[0][:], xst[:, :, ts(0, QS)])
        nc.sync.dma_start(xot_sb[:], xot)
        nc.sync.dma_start(wq_sb[:], Wq)
        for b in range(1, 8):
            nc.sync.dma_start(xs_tiles[b][:], xst[:, :, ts(b, QS)])
        nc.sync.dma_start(wo_sb[:], Wo)
        nc.sync.dma_start(bo_sb[:], bo)

        with (
            tc.tile_pool(name="scores_ps", bufs=3, space="PSUM") as scoresp,
            tc.tile_pool(name="ctx_ps", bufs=2, space="PSUM") as ctxp,
            tc.tile_pool(name="attn", bufs=21) as attnp,
            tc.tile_pool(name="norm", bufs=2) as normp,
            tc.tile_pool(name="cs", bufs=4) as csp,
            tc.tile_pool(name="odd", bufs=2) as oddp,
            tc.tile_pool(name="out_sb", bufs=2) as outsb,
        ):
            # replicated v for block b: 4 chunks of [128 rows, 4 heads x 64]
            def vproj(b):
                for m in range(4):
                    ps = scoresp.tile([128, 1024], F32, tag="sc", name="ps")
                    nc.tensor.matmul(
                        ps[:, 0:KV], ones_sb[0:1, 0:128], bv_sb[0:1, :],
                        start=True, stop=False,
                    )
                    for cc in range(CC):
                        nc.tensor.matmul(
                            ps[:, 0:KV],
                            xs_tiles[b][:, cc, ts(m, 128)], wv_sb[:, cc, :],
                            start=False, stop=(cc == CC - 1),
                        )
                    src = ps[:, 0:KV].rearrange("p (g d) -> p g d", g=4)
                    dst = v_sbs[b][:, m // 2, m % 2, :, 0:HD]
                    if m % 2 == 0:
                        nc.scalar.copy(dst, src)
                    else:
                        nc.vector.tensor_copy(out=dst, in_=src)

            # replicated kT for block b of the sequence
            def kproj(b):
                for j in range(2):
                    ps = scoresp.tile([128, 1024], F32, tag="sc", name="ps")
                    for cc in range(CC):
                        nc.tensor.matmul(
                            ps[:, 0:512],
                            wk_sb[:, cc, ts(j, 128)], xs_tiles[b][:, cc, :],
                            start=(cc == 0), stop=(cc == CC - 1),
                        )
                    ktg = kt0_sb if j == 0 else kt1_sb
                    if j == 0:
                        nc.vector.tensor_scalar(
                            out=ktg[:, ts(b, QS)], in0=ps[:, 0:512],
                            scalar1=bk_sb[:, 0:1], scalar2=None,
                            op0=mybir.AluOpType.add,
                        )
                    else:
                        nc.scalar.add(ktg[:, ts(b, QS)], ps[:, 0:512], bk_sb[:, 1:2])

            def qproj(s):
                ps = scoresp.tile([128, 1024], F32, tag="sc", name="ps")
                for cc in range(CC):
                    nc.tensor.matmul(
                        ps[:, 0:512],
                        wq_sb[:, cc, ts(s, 128)], xot_sb[:, cc, :],
                        start=(cc == 0), stop=(cc == CC - 1),
                    )
                if s % 2 == 0:
                    nc.vector.tensor_scalar(
                        out=qt_sb[:, s, :], in0=ps[:, 0:512],
                        scalar1=bq_sb[:, s : s + 1], scalar2=None,
                        op0=mybir.AluOpType.add,
                    )
                else:
                    nc.scalar.add(qt_sb[:, s, :], ps[:, 0:512], bq_sb[:, s : s + 1])

            # ---- attention: global cross-slot AV pipeline ----
            # AVs pop strictly in (slot, kcp) order even though slot 0/1
            # scores interleave in the front region: only one slot's ctx
            # PSUM pair is ever live (the 2-buf ring), and slot s+1's first
            # AV emits after slot s's normalize, so the ring reuse dep
            # always points backwards in the PE queue (no deadlock).
            ctx_of = {}
            pend = {s: [] for s in range(8)}
            pend_n = [0]
            cur_s = [0]

            def normalize(s, ctx_a, ctx_b):
                # Spill ctx PSUM -> SBUF right away (ScalarE head a,
                # VectorE head b) so the 2-deep ctx PSUM ring frees fast.
                cs_a = csp.tile([HD + 1, QS], F32, tag="cs", name="cs_a")
                nc.scalar.copy(cs_a[:], ctx_a[:])
                cs_b = csp.tile([HD + 1, QS], F32, tag="cs", name="cs_b")
                nc.vector.tensor_copy(out=cs_b[:], in_=ctx_b[:])

                # normalize: ctxT[d, q] * (1/denom[q]); head a -> parts
                # 0:64, head b -> 64:128 via sb2sb DMA partition shift.
                dn_a = normp.tile([1, QS], F32, tag="dn", name="dn_a")
                nc.sync.dma_start(dn_a[:], cs_a[HD : HD + 1, :])
                db_a = normp.tile([64, QS], F32, tag="db", name="db_a")
                nc.gpsimd.partition_broadcast(db_a[:], dn_a[:], channels=64)
                rb_a = normp.tile([64, QS], F32, tag="rbcast", name="rb_a")
                nc.vector.reciprocal_approx_fast(rb_a[:], db_a[:])
                nc.gpsimd.tensor_tensor(
                    out=ctxt_sbs[s][0:64, :], in0=cs_a[0:HD, :], in1=rb_a[:],
                    op=mybir.AluOpType.mult,
                )

                dn_b = normp.tile([1, QS], F32, tag="dn", name="dn_b")
                nc.sync.dma_start(dn_b[:], cs_b[HD : HD + 1, :])
                db_b = normp.tile([64, QS], F32, tag="db", name="db_b")
                nc.gpsimd.partition_broadcast(db_b[:], dn_b[:], channels=64)
                rb_b = normp.tile([64, QS], F32, tag="rbcast", name="rb_b")
                nc.vector.reciprocal_approx_fast(rb_b[:], db_b[:])
                tmp = oddp.tile([64, QS], F16, tag="odd", name="tmp")
                nc.gpsimd.tensor_tensor(
                    out=tmp[:], in0=cs_b[0:HD, :], in1=rb_b[:],
                    op=mybir.AluOpType.mult,
                )
                nc.sync.dma_start(ctxt_sbs[s][64:128, :], tmp[:])

            def pop_av():
                s = cur_s[0]
                kcp, at2 = pend[s].pop(0)
                pend_n[0] -= 1
                if s not in ctx_of:
                    ctx_of[s] = (
                        ctxp.tile([HD + 1, QS], F32, tag="ctx", name=f"ctxa{s}"),
                        ctxp.tile([HD + 1, QS], F32, tag="ctx", name=f"ctxb{s}"),
                    )
                ctx_a, ctx_b = ctx_of[s]
                g2 = 0 if s < 4 else 2
                vb = v_sbs[kcp // 2]
                kk = kcp % 2
                nc.tensor.matmul(
                    ctx_a[:], vb[:, kk, :, g2, 0 : HD + 1],
                    at2[:, :, 0:512],
                    perf_mode=mybir.MatmulPerfMode.DoubleRow,
                    start=(kcp == 0), stop=(kcp == KC // 2 - 1),
                    skip_group_check=True,
                )
                nc.tensor.matmul(
                    ctx_b[:], vb[:, kk, :, g2 + 1, 0 : HD + 1],
                    at2[:, :, 512:1024],
                    perf_mode=mybir.MatmulPerfMode.DoubleRow,
                    start=(kcp == 0), stop=(kcp == KC // 2 - 1),
                    skip_group_check=True,
                )
                if kcp == KC // 2 - 1:
                    normalize(s, ctx_a, ctx_b)
                    del ctx_of[s]
                    cur_s[0] = s + 1

            def scores_exp(s, kcp):
                ktg = kt0_sb if s < 4 else kt1_sb
                at2 = attnp.tile([128, 2, 1024], F8, tag="at", name=f"at2_{s}_{kcp}")
                for j in range(2):
                    kc = 2 * kcp + j
                    sc = scoresp.tile([128, 1024], F32, tag="sc", name="sc")
                    nc.tensor.matmul(
                        sc[:, 0:512],
                        ktg[0:64, ts(kc, 128)], qt_sb[0:64, s, :],
                        start=True, stop=True,
                    )
                    nc.tensor.matmul(
                        sc[:, 512:1024],
                        ktg[64:128, ts(kc, 128)], qt_sb[64:128, s, :],
                        start=True, stop=True,
                    )
                    if DVE_TILE[kc]:
                        # DVE fast-exp2 -> fp8 bits in ONE op
                        nc.vector.tensor_scalar(
                            out=at2[:, j, :].bitcast(I8), in0=sc[:],
                            scalar1=EXP_SCALE8, scalar2=EXP_OFFSET8,
                            op0=mybir.AluOpType.mult,
                            op1=mybir.AluOpType.add,
                        )
                    else:
                        nc.scalar.activation(
                            at2[:, j, :], sc[:],
                            mybir.ActivationFunctionType.Exp, scale=0.125,
                        )
                pend[s].append((kcp, at2))
                pend_n[0] += 1
                pops = 0
                while (
                    pend_n[0] > AV_LAG and pops < 2
                    and cur_s[0] < 8 and pend[cur_s[0]]
                ):
                    pop_av()
                    pops += 1

            # ---- front region: projections interleaved with slots 0-1 ----
            kproj(0)
            qproj(0)
            qproj(1)
            for b in range(8):
                if b > 0:
                    kproj(b)
                vproj(b)
                scores_exp(0, 2 * b)
                scores_exp(0, 2 * b + 1)
                scores_exp(1, 2 * b)
                scores_exp(1, 2 * b + 1)
            qproj(2)

            # ---- steady slots 2-7 (next slot's qproj rides PE slack) ----
            for s in range(2, 8):
                for kcp in range(KC // 2):
                    if kcp == 8 and s < 7:
                        qproj(s + 1)
                    scores_exp(s, kcp)
            while pend_n[0] > 0:
                pop_av()

            # ---- output projection (per-slot ctxt deps let s=0..6
            # accumulate while slot 7 finishes normalizing) ----
            for qt in range(QT):
                po = scoresp.tile([128, 1024], F32, tag="sc", name="po")
                for half in range(2):
                    nc.tensor.matmul(
                        po[:, ts(half, 512)],
                        ones_sb[0:1, 0:128], bo_sb[0:1, ts(half, 512)],
                        start=True, stop=False,
                    )
                    for s in range(8):
                        nc.tensor.matmul(
                            po[:, ts(half, 512)],
                            ctxt_sbs[s][:, ts(qt, 128)],
                            wo_sb[:, s, ts(half, 512)],
                            start=False, stop=(s == 7),
                        )
                ob = outsb.tile([128, DM], F32, tag="ob", name="ob")
                if qt % 2 == 0:
                    nc.vector.tensor_copy(out=ob[:], in_=po[:])
                else:
                    nc.scalar.copy(ob[:], po[:])
                nc.sync.dma_start(out[ts(qt, 128), :], ob[:])


def build():
    if "nc" in _CACHE:
        return _CACHE["nc"]
    nc = bacc.Bacc(
        "TRN2", target_bir_lowering=False, debug=False, num_devices=N_CORES
    )
    with tile.TileContext(nc) as tc:
        _emit(tc)
    nc.compile()
    _CACHE["nc"] = nc
    return nc


def make_in_maps(inputs) -> list[dict]:
    """Host-side staging: cast to fp16 and pre-shuffle into SBUF layouts."""
    x = np.asarray(inputs["x"], dtype=np.float32).reshape(SEQ, DM)
    Wq = np.asarray(inputs["Wq"], dtype=np.float32).reshape(DM, DM)
    bq = np.asarray(inputs["bq"], dtype=np.float32).reshape(DM)
    Wk = np.asarray(inputs["Wk"], dtype=np.float32).reshape(DM, KV)
    bk = np.asarray(inputs["bk"], dtype=np.float32).reshape(KV)
    Wv = np.asarray(inputs["Wv"], dtype=np.float32).reshape(DM, KV)
    bv = np.asarray(inputs["bv"], dtype=np.float32).reshape(KV)
    Wo = np.asarray(inputs["Wo"], dtype=np.float32).reshape(DM, DM)
    bo = np.asarray(inputs["bo"], dtype=np.float32).reshape(DM)

    # x^T as [p, cc, seq]
    xt16 = np.ascontiguousarray(
        x.T.reshape(CC, 128, SEQ).transpose(1, 0, 2).astype(np.float16)
    )
    # Wk/Wv as [p, cc, kv]
    wk16 = np.ascontiguousarray(
        Wk.reshape(CC, 128, KV).transpose(1, 0, 2).astype(np.float16)
    )
    wv16 = np.ascontiguousarray(
        Wv.reshape(CC, 128, KV).transpose(1, 0, 2).astype(np.float16)
    )
    # Wq shuffled: slot s = 4*g2+i holds q-head pair (8*g2+i, 8*g2+i+4);
    # model col for (s, half, d) is 512*g2 + 256*half + 64*i + d.
    Wqr = Wq.reshape(CC, 128, DM)
    wq16 = np.zeros((128, CC, DM), np.float16)
    bq16 = np.zeros((1, DM), np.float16)
    wo16 = np.zeros((128, CC, DM), np.float16)
    for g2 in range(2):
        for i in range(4):
            s = 4 * g2 + i
            for h in range(2):
                col = 512 * g2 + 256 * h + 64 * i
                dst = 128 * s + 64 * h
                wq16[:, :, dst : dst + 64] = Wqr[:, :, col : col + 64].transpose(
                    1, 0, 2
                )
                bq16[0, dst : dst + 64] = bq[col : col + 64]
                wo16[64 * h : 64 * h + 64, s, :] = Wo[col : col + 64, :]
    shared = {
        "xst": xt16,
        "wq": wq16,
        "bq2": np.ascontiguousarray(bq16[0].reshape(CC, 128).T.astype(np.float32)),
        "wk": wk16,
        "bk2": np.ascontiguousarray(bk.astype(np.float32).reshape(2, 128).T),
        "wv": wv16,
        "bv": bv.reshape(1, KV).astype(np.float16),
        "wo": wo16,
        "bo": bo.reshape(1, DM).astype(np.float16),
    }
    return [
        dict(
            shared,
            xot=np.ascontiguousarray(xt16[:, :, c * QS : (c + 1) * QS]),
        )
        for c in range(N_CORES)
    ]


def kernel(**inputs) -> np.ndarray:
    nc = build()
    in_maps = make_in_maps(inputs)
    res = run_bass_kernel_spmd(nc, in_maps, core_ids=list(range(N_CORES)))
    full = np.concatenate([res.results[c]["out"] for c in range(N_CORES)], axis=0)
    return full[None].astype(np.float32)


if __name__ == "__main__":
    rng = np.random.default_rng(0)
    s = 0.02
    inputs = {
        "x": rng.standard_normal((1, SEQ, DM), dtype=np.float32),
        "Wq": rng.standard_normal((DM, DM), dtype=np.float32) * s,
        "bq": rng.standard_normal((DM,), dtype=np.float32) * s,
        "Wk": rng.standard_normal((DM, KV), dtype=np.float32) * s,
        "bk": rng.standard_normal((KV,), dtype=np.float32) * s,
        "Wv": rng.standard_normal((DM, KV), dtype=np.float32) * s,
        "bv": rng.standard_normal((KV,), dtype=np.float32) * s,
        "Wo": rng.standard_normal((DM, DM), dtype=np.float32) * s,
        "bo": rng.standard_normal((DM,), dtype=np.float32) * s,
    }
    out = kernel(**inputs)
    print("out shape", out.shape, "finite", np.isfinite(out).all())
